# revision 34
# baseline (speedup 1.0000x reference)
"""RBF-kernel SVM inference on 8 Trainium2 NeuronCores.

out[m] = sum_n w[n] * exp(-g * ||x[m] - xt[n]||^2) + b
       = exp(-g*||x[m]||^2) * sum_n w[n] * exp(2g*x[m].xt[n] - g*||xt[n]||^2) + b

Sharding: rows of x split across 8 cores (1024 each); x_train / weight
replicated.

Per core, the [8192, 1024] kernel slab is produced as 32 "pairs" of
128-wide n-tiles: 8 fp8 DoubleRow matmuls (K=512) fill a [128, 2, 1024]
PSUM tile, one 2048-wide Exp on ScalarE produces fp16
u = |w_n| * exp(2g*x.xt_n - g*||xt_n||^2)  (ln|w| and -g*||xt||^2 are
both folded into the per-partition activation bias), and a single
VectorE tensor add/subtract accumulates the pair into acc2. The
n-points are sorted host-side by (sign(w), combined bias) so that (a)
each pair shares one bias vector — enabling the 2048-wide activation —
and (b) every pair is sign-pure (one mixed boundary pair uses
scalar_tensor_tensor with a +-1 column instead). The final 128-way
cross-partition sum is eight K=128 matmuls (the last pair's u planes
feed theirs directly, signs as the stationary operand, keeping the
last VectorE op off the tail); the host applies exp(-g*||x||^2) and
the bias to the returned sums.

ScalarE is the bottleneck engine (32 x 1.97us back-to-back Exps =
63us of the ~84us total); PE (~57us) and DVE (~41us) hide under it.
Junk matmuls at the head warm the PE HAM clock gate while the input
DMAs land on all three DGE rings.
"""

import os
import sys

for _p in ("/opt/trn_rl_repo", "/root/.axon_site/_ro/trn_rl_repo"):
    if os.path.isdir(_p) and _p not in sys.path:
        sys.path.append(_p)

import numpy as np
import ml_dtypes

import concourse.bass as bass
import concourse.mybir as mybir
import concourse.tile as tile
from concourse import bacc
from concourse.bass_utils import run_bass_kernel_spmd

M, N, D = 8192, 8192, 512
NCORES = 8
MC = M // NCORES  # rows of x per core

LAST_RESULTS = None  # BassKernelResults of the most recent run (for test.py)

# x_train DMA groups (n-tile counts): tiny leading groups so the first
# matmuls can start after ~128KB, then big single-dispatch chunks.
GROUPS = (1, 1, 6, 8, 16, 32)
PREWARM = 6   # junk N=512 matmuls at the head to lift the PE HAM clock gate


def build(pair_ops, mc=MC, n=N, d=D, ncores=NCORES):
    """Build + compile the per-core program.

    pair_ops: tuple of 32 chars, one per n-tile pair: 'a' = acc2 += u,
    's' = acc2 -= u, 'm' = mixed-sign pair (uses the sg +-1 columns).
    """
    P = 128
    KT = d // P            # K tiles in the contraction (4)
    NT = n // P            # n tiles (64)
    NPAIR = NT // 2        # activation pairs (32)
    KP = KT // 2           # DoubleRow passes (K=256 each)
    MCH = 512              # m chunk = one PSUM bank of f32
    assert len(pair_ops) == NPAIR

    f32 = mybir.dt.float32
    bf16 = mybir.dt.bfloat16
    f16 = mybir.dt.float16
    f8 = mybir.dt.float8e4

    nc = bacc.Bacc(
        "TRN2",
        target_bir_lowering=False,
        debug=False,
        enable_asserts=False,
        num_devices=ncores,
    )

    # Host-prepared layouts: leading dim is the SBUF partition.
    xt_d = nc.dram_tensor("xt", (P, KT, mc), f8, kind="ExternalInput")
    bt_d = nc.dram_tensor("bt", (P, NT, KT, P), f8, kind="ExternalInput")
    bn_d = nc.dram_tensor("bn", (P, NPAIR), f32, kind="ExternalInput")
    sg_d = nc.dram_tensor("sg", (P, 2), f32, kind="ExternalInput")
    s31_d = nc.dram_tensor("s31", (P, 2), f16, kind="ExternalInput")
    out_d = nc.dram_tensor("out", (1, mc), f32, kind="ExternalOutput")

    EXP = mybir.ActivationFunctionType.Exp
    MUL = mybir.AluOpType.mult
    ADD = mybir.AluOpType.add

    gstart = np.cumsum([0] + list(GROUPS))

    with tile.TileContext(nc) as tc:
        with (
            tc.tile_pool(name="const", bufs=1) as const,
            tc.tile_pool(name="bt_pool", bufs=1) as bt_pool,
            tc.tile_pool(name="e_pool", bufs=3) as e_pool,
            tc.tile_pool(name="pt_pool", bufs=2, space="PSUM") as pt_pool,
        ):
            bn_sb = const.tile([P, NPAIR], f32, name="bn_sb")
            sg_sb = const.tile([P, 2], f32, name="sg_sb")
            s31_sb = const.tile([P, 2], f16, name="s31_sb")
            xt_sb = const.tile([P, KT, mc], f8, name="xt_sb")
            ones = const.tile([P, 1], f16, name="ones")
            acc2 = const.tile([P, 2, mc], f16, name="acc2")
            jw = const.tile([P, 2, P], f8, name="jw")
            jm = const.tile([P, 2, MCH], f8, name="jm")

            # DMA dispatches first so transfers start immediately: the x
            # slab is quartered across the SP and Activation HW-DGE rings
            # (K-pass 0 quarters lead), x_train streams on the gpsimd
            # software DGE. One early dispatch on the Scalar queue is
            # harmless — activations start much later.
            nc.sync.dma_start(xt_sb[:, 0:1, :], xt_d[:, 0:1, :])
            nc.scalar.dma_start(xt_sb[:, 1:2, :], xt_d[:, 1:2, :])
            nc.sync.dma_start(xt_sb[:, 2:3, :], xt_d[:, 2:3, :])
            nc.scalar.dma_start(xt_sb[:, 3:4, :], xt_d[:, 3:4, :])
            nc.sync.dma_start(bn_sb[:], bn_d[:])
            nc.sync.dma_start(sg_sb[:], sg_d[:])
            nc.sync.dma_start(s31_sb[:], s31_d[:])

            bt_sb = []
            for gi, gn in enumerate(GROUPS):
                t = bt_pool.tile([P, gn, KT, P], f8, name=f"bt_sb{gi}")
                bt_sb.append(t)
                nc.gpsimd.dma_start(
                    t[:], bt_d[:, gstart[gi]:gstart[gi + 1], :, :]
                )

            # PE warm-up on junk operands while the DMAs land: keeps the
            # HAM activity window busy so the real matmul stream starts at
            # 2.4 GHz instead of 1.2.
            nc.vector.memset(jw[:], 0.0)
            nc.vector.memset(jm[:], 0.0)
            nc.vector.memset(ones[:], 1.0)
            nc.vector.memset(acc2[:], 0.0)
            if PREWARM:
                ptw = pt_pool.tile([P, 2, mc], f32, name="ptw", tag="pt")
                for r in range(PREWARM):
                    nc.tensor.matmul(
                        ptw[:, 0, 0:MCH], jw[:], jm[:],
                        perf_mode=mybir.MatmulPerfMode.DoubleRow,
                    )

            def bt_slice(t, p):
                """Stationary operand for n-tile t, DoubleRow pass p."""
                gi = int(np.searchsorted(gstart, t, side="right")) - 1
                lc = t - gstart[gi]
                return bt_sb[gi][:, lc, 2 * p:2 * p + 2, :]

            u_last = None
            for i in range(NPAIR):
                pt2 = pt_pool.tile([P, 2, mc], f32, name="pt2", tag="pt")
                for p in range(KP):
                    for h in range(2):
                        t = 2 * i + h
                        for j in range(mc // MCH):
                            nc.tensor.matmul(
                                pt2[:, h, j * MCH:(j + 1) * MCH],
                                bt_slice(t, p),
                                xt_sb[:, 2 * p:2 * p + 2, j * MCH:(j + 1) * MCH],
                                start=(p == 0),
                                stop=(p == KP - 1),
                                perf_mode=mybir.MatmulPerfMode.DoubleRow,
                            )
                u = e_pool.tile([P, 2, mc], f16, name="u")
                nc.scalar.activation(u[:], pt2[:], EXP, bias=bn_sb[:, i:i + 1])
                op = pair_ops[i]
                if op == "f":
                    # Last pair: consumed directly by the final matmuls
                    # (signs come in as the s31 stationary columns), keeping
                    # the VectorE accumulate off the tail critical path.
                    u_last = u
                elif op == "a":
                    nc.vector.tensor_add(acc2[:], acc2[:], u[:])
                elif op == "s":
                    nc.vector.tensor_sub(acc2[:], acc2[:], u[:])
                else:  # mixed-sign boundary pair
                    for h in range(2):
                        nc.vector.scalar_tensor_tensor(
                            acc2[:, h, :], u[:, h, :], sg_sb[:, h:h + 1],
                            acc2[:, h, :], MUL, ADD,
                        )

            # 128-way cross-partition sum into ptk's upper plane (ptk
            # recycles the pool slot last read by ACT pair 30, so the
            # acc2 sums run DURING the last activation; the last pair's u
            # feeds its two matmuls directly, gated only on that
            # activation). S = ones.T @ acc2 planes + s31.T @ u_last
            # planes, accumulated per chunk; host applies exp(-g*||x||^2)
            # and the bias.
            ptk = pt_pool.tile([P, 2, mc], f32, name="ptk", tag="pt")
            fin = const.tile([1, mc], f32, name="fin")
            for j in range(mc // MCH):
                sl = slice(j * MCH, (j + 1) * MCH)
                for h in range(2):
                    nc.tensor.matmul(
                        ptk[0:1, 1, sl], ones[:], acc2[:, h, sl],
                        start=(h == 0), stop=False,
                    )
                for h in range(2):
                    nc.tensor.matmul(
                        ptk[0:1, 1, sl], s31_sb[:, h:h + 1], u_last[:, h, sl],
                        start=False, stop=(h == 1),
                    )
                if j == 0:
                    nc.vector.tensor_copy(fin[:, sl], ptk[0:1, 1, sl])
                else:
                    nc.scalar.copy(fin[:, sl], ptk[0:1, 1, sl])
            nc.sync.dma_start(out_d[:], fin[:])

    nc.compile()
    return nc


_CACHE = {}


def _get_nc(pair_ops):
    if pair_ops not in _CACHE:
        _CACHE[pair_ops] = build(pair_ops)
    return _CACHE[pair_ops]


def kernel(x, x_train, gamma, weight, bias):
    global LAST_RESULTS
    x = np.asarray(x, dtype=np.float32)
    x_train = np.asarray(x_train, dtype=np.float32)
    g = float(np.asarray(gamma).reshape(-1)[0])
    w = np.asarray(weight, dtype=np.float32).reshape(N)
    b = np.float32(np.asarray(bias).reshape(-1)[0])

    P = 128
    KT = D // P
    NT = N // P
    NPAIR = NT // 2
    xx = np.einsum("md,md->m", x, x, dtype=np.float64, optimize=True)
    yy = np.einsum("nd,nd->n", x_train, x_train, dtype=np.float64, optimize=True)

    # Combined per-n exp offset: c_n = -g*||xt_n||^2 + ln|w_n|. Sort n by
    # (sign(w), c) and interleave so activation pair i, partition p holds
    # sorted slots 256i+2p (tile 2i) and 256i+2p+1 (tile 2i+1): pairs
    # share a midpoint bias and are sign-pure except one boundary pair.
    c = -g * yy + np.log(np.maximum(np.abs(w), 1e-30))
    neg = (w < 0).astype(np.int64)
    order = np.lexsort((c, neg))  # positives (by c), then negatives (by c)
    npos = int((neg == 0).sum())

    slot = order.reshape(NPAIR, P, 2)             # [pair, partition, h]
    idx = slot.transpose(0, 2, 1).reshape(NT, P)  # [tile, partition]

    bn = np.ascontiguousarray(
        c[slot].mean(axis=2).T.astype(np.float32)  # [128, NPAIR]
    )

    # Per-pair accumulate ops + sign columns for the mixed boundary pair.
    # The last pair is always 'f': its u planes feed the final matmuls
    # directly with the s31 sign columns as the stationary operand.
    pair_ops = []
    bp = npos // 256
    for i in range(NPAIR):
        if i == NPAIR - 1:
            pair_ops.append("f")
        elif (i + 1) * 256 <= npos:
            pair_ops.append("a")
        elif i * 256 >= npos:
            pair_ops.append("s")
        else:
            pair_ops.append("m")
    pair_ops = tuple(pair_ops)
    sg = np.ones((P, 2), dtype=np.float32)
    if "m" in pair_ops:
        sg = np.where(w[slot[bp]] >= 0, 1.0, -1.0).astype(np.float32)
    sg = np.ascontiguousarray(sg)
    s31 = np.ascontiguousarray(
        np.where(w[slot[NPAIR - 1]] >= 0, 1.0, -1.0).astype(np.float16)
    )

    # fp8 operand layouts, partition-major for single-dispatch DMA:
    # arr[p, k, cols] = src[k*128 + p, cols].
    xt_t = np.ascontiguousarray(x.T).astype(ml_dtypes.float8_e4m3)  # [D, M]
    xt_r = xt_t.reshape(KT, P, M).transpose(1, 0, 2)                # [P, KT, M]
    bt_t = ((2.0 * g) * x_train[idx.reshape(-1)].T).astype(ml_dtypes.float8_e4m3)
    # [P, NT, KT, P]: per x_train-group DMA is fully contiguous per partition
    bt_r = np.ascontiguousarray(
        bt_t.reshape(KT, P, NT, P).transpose(1, 2, 0, 3)
    )

    in_maps = []
    for cix in range(NCORES):
        sl = slice(cix * MC, (cix + 1) * MC)
        in_maps.append({
            "xt": np.ascontiguousarray(xt_r[:, :, sl]),
            "bt": bt_r,
            "bn": bn,
            "sg": sg,
            "s31": s31,
        })

    nc = _get_nc(pair_ops)
    res = run_bass_kernel_spmd(nc, in_maps, core_ids=list(range(NCORES)))
    LAST_RESULTS = res
    s = np.concatenate(
        [np.asarray(res.results[c]["out"], dtype=np.float32).reshape(MC) for c in range(NCORES)]
    )
    # Epilogue: out = S * exp(-g*||x||^2) + b
    out = (s * np.exp(-g * xx)).astype(np.float32) + b
    return out.astype(np.float32).reshape(M, 1)


# revision 35
# speedup vs baseline: 1.0000x; 1.0000x over previous
"""RBF-kernel SVM inference on 8 Trainium2 NeuronCores.

out[m] = sum_n w[n] * exp(-g * ||x[m] - xt[n]||^2) + b
       = exp(-g*||x[m]||^2) * sum_n w[n] * exp(2g*x[m].xt[n] - g*||xt[n]||^2) + b

Sharding: rows of x split across 8 cores (1024 each); x_train / weight
replicated.

Per core, the [8192, 1024] kernel slab is produced as 32 "pairs" of
128-wide n-tiles: 8 fp8 DoubleRow matmuls (K=512) fill a [128, 2, 1024]
PSUM tile, one 2048-wide Exp on ScalarE produces fp16
u = |w_n| * exp(2g*x.xt_n - g*||xt_n||^2)  (ln|w| and -g*||xt||^2 are
both folded into the per-partition activation bias), and a single
VectorE tensor add/subtract accumulates the pair into acc2. The
n-points are sorted host-side by (sign(w), combined bias) so that (a)
each pair shares one bias vector — enabling the 2048-wide activation —
and (b) every pair is sign-pure (one mixed boundary pair uses
scalar_tensor_tensor with a +-1 column instead). The final 128-way
cross-partition sum is eight K=128 matmuls (the last pair's u planes
feed theirs directly, signs as the stationary operand, keeping the
last VectorE op off the tail); the host applies exp(-g*||x||^2) and
the bias to the returned sums.

ScalarE is the bottleneck engine (32 x 1.97us back-to-back Exps =
63us of the ~84us total); PE (~57us) and DVE (~41us) hide under it.
Junk matmuls at the head warm the PE HAM clock gate while the input
DMAs land on all three DGE rings.
"""

import os
import sys

for _p in ("/opt/trn_rl_repo", "/root/.axon_site/_ro/trn_rl_repo"):
    if os.path.isdir(_p) and _p not in sys.path:
        sys.path.append(_p)

import numpy as np
import ml_dtypes

import concourse.bass as bass
import concourse.mybir as mybir
import concourse.tile as tile
from concourse import bacc
from concourse.bass_utils import run_bass_kernel_spmd

M, N, D = 8192, 8192, 512
NCORES = 8
MC = M // NCORES  # rows of x per core

LAST_RESULTS = None  # BassKernelResults of the most recent run (for test.py)

# x_train DMA groups (n-tile counts): tiny leading groups so the first
# matmuls can start after ~128KB, then big single-dispatch chunks.
GROUPS = (2, 6, 8, 16, 32)
PREWARM = 6   # junk N=512 matmuls at the head to lift the PE HAM clock gate


def build(pair_ops, mc=MC, n=N, d=D, ncores=NCORES):
    """Build + compile the per-core program.

    pair_ops: tuple of 32 chars, one per n-tile pair: 'a' = acc2 += u,
    's' = acc2 -= u, 'm' = mixed-sign pair (uses the sg +-1 columns).
    """
    P = 128
    KT = d // P            # K tiles in the contraction (4)
    NT = n // P            # n tiles (64)
    NPAIR = NT // 2        # activation pairs (32)
    KP = KT // 2           # DoubleRow passes (K=256 each)
    MCH = 512              # m chunk = one PSUM bank of f32
    assert len(pair_ops) == NPAIR

    f32 = mybir.dt.float32
    bf16 = mybir.dt.bfloat16
    f16 = mybir.dt.float16
    f8 = mybir.dt.float8e4

    nc = bacc.Bacc(
        "TRN2",
        target_bir_lowering=False,
        debug=False,
        enable_asserts=False,
        num_devices=ncores,
    )

    # Host-prepared layouts: leading dim is the SBUF partition.
    xt_d = nc.dram_tensor("xt", (P, KT, mc), f8, kind="ExternalInput")
    bt_d = nc.dram_tensor("bt", (P, NT, KT, P), f8, kind="ExternalInput")
    bn_d = nc.dram_tensor("bn", (P, NPAIR), f32, kind="ExternalInput")
    sg_d = nc.dram_tensor("sg", (P, 2), f32, kind="ExternalInput")
    s31_d = nc.dram_tensor("s31", (P, 2), f16, kind="ExternalInput")
    out_d = nc.dram_tensor("out", (1, mc), f32, kind="ExternalOutput")

    EXP = mybir.ActivationFunctionType.Exp
    MUL = mybir.AluOpType.mult
    ADD = mybir.AluOpType.add

    gstart = np.cumsum([0] + list(GROUPS))

    with tile.TileContext(nc) as tc:
        with (
            tc.tile_pool(name="const", bufs=1) as const,
            tc.tile_pool(name="bt_pool", bufs=1) as bt_pool,
            tc.tile_pool(name="e_pool", bufs=3) as e_pool,
            tc.tile_pool(name="pt_pool", bufs=2, space="PSUM") as pt_pool,
        ):
            bn_sb = const.tile([P, NPAIR], f32, name="bn_sb")
            sg_sb = const.tile([P, 2], f32, name="sg_sb")
            s31_sb = const.tile([P, 2], f16, name="s31_sb")
            xt_sb = const.tile([P, KT, mc], f8, name="xt_sb")
            ones = const.tile([P, 1], f16, name="ones")
            acc2 = const.tile([P, 2, mc], f16, name="acc2")
            jw = const.tile([P, 2, P], f8, name="jw")
            jm = const.tile([P, 2, MCH], f8, name="jm")

            # DMA dispatches first so transfers start immediately: the x
            # slab is quartered across the SP and Activation HW-DGE rings
            # (K-pass 0 quarters lead), x_train streams on the gpsimd
            # software DGE. One early dispatch on the Scalar queue is
            # harmless — activations start much later.
            nc.sync.dma_start(xt_sb[:, 0:1, :], xt_d[:, 0:1, :])
            nc.scalar.dma_start(xt_sb[:, 1:2, :], xt_d[:, 1:2, :])
            nc.sync.dma_start(xt_sb[:, 2:3, :], xt_d[:, 2:3, :])
            nc.scalar.dma_start(xt_sb[:, 3:4, :], xt_d[:, 3:4, :])
            nc.sync.dma_start(bn_sb[:], bn_d[:])
            nc.sync.dma_start(sg_sb[:], sg_d[:])
            nc.sync.dma_start(s31_sb[:], s31_d[:])

            bt_sb = []
            for gi, gn in enumerate(GROUPS):
                t = bt_pool.tile([P, gn, KT, P], f8, name=f"bt_sb{gi}")
                bt_sb.append(t)
                nc.gpsimd.dma_start(
                    t[:], bt_d[:, gstart[gi]:gstart[gi + 1], :, :]
                )

            # PE warm-up on junk operands while the DMAs land: keeps the
            # HAM activity window busy so the real matmul stream starts at
            # 2.4 GHz instead of 1.2.
            nc.vector.memset(jw[:], 0.0)
            nc.vector.memset(jm[:], 0.0)
            nc.vector.memset(ones[:], 1.0)
            nc.vector.memset(acc2[:], 0.0)
            if PREWARM:
                ptw = pt_pool.tile([P, 2, mc], f32, name="ptw", tag="pt")
                for r in range(PREWARM):
                    nc.tensor.matmul(
                        ptw[:, 0, 0:MCH], jw[:], jm[:],
                        perf_mode=mybir.MatmulPerfMode.DoubleRow,
                    )

            def bt_slice(t, p):
                """Stationary operand for n-tile t, DoubleRow pass p."""
                gi = int(np.searchsorted(gstart, t, side="right")) - 1
                lc = t - gstart[gi]
                return bt_sb[gi][:, lc, 2 * p:2 * p + 2, :]

            u_last = None
            for i in range(NPAIR):
                pt2 = pt_pool.tile([P, 2, mc], f32, name="pt2", tag="pt")
                for p in range(KP):
                    for h in range(2):
                        t = 2 * i + h
                        for j in range(mc // MCH):
                            nc.tensor.matmul(
                                pt2[:, h, j * MCH:(j + 1) * MCH],
                                bt_slice(t, p),
                                xt_sb[:, 2 * p:2 * p + 2, j * MCH:(j + 1) * MCH],
                                start=(p == 0),
                                stop=(p == KP - 1),
                                perf_mode=mybir.MatmulPerfMode.DoubleRow,
                            )
                u = e_pool.tile([P, 2, mc], f16, name="u")
                nc.scalar.activation(u[:], pt2[:], EXP, bias=bn_sb[:, i:i + 1])
                op = pair_ops[i]
                if op == "f":
                    # Last pair: consumed directly by the final matmuls
                    # (signs come in as the s31 stationary columns), keeping
                    # the VectorE accumulate off the tail critical path.
                    u_last = u
                elif op == "a":
                    nc.vector.tensor_add(acc2[:], acc2[:], u[:])
                elif op == "s":
                    nc.vector.tensor_sub(acc2[:], acc2[:], u[:])
                else:  # mixed-sign boundary pair
                    for h in range(2):
                        nc.vector.scalar_tensor_tensor(
                            acc2[:, h, :], u[:, h, :], sg_sb[:, h:h + 1],
                            acc2[:, h, :], MUL, ADD,
                        )

            # 128-way cross-partition sum into ptk's upper plane (ptk
            # recycles the pool slot last read by ACT pair 30, so the
            # acc2 sums run DURING the last activation; the last pair's u
            # feeds its two matmuls directly, gated only on that
            # activation). S = ones.T @ acc2 planes + s31.T @ u_last
            # planes, accumulated per chunk; host applies exp(-g*||x||^2)
            # and the bias.
            ptk = pt_pool.tile([P, 2, mc], f32, name="ptk", tag="pt")
            fin = const.tile([1, mc], f32, name="fin")
            for j in range(mc // MCH):
                sl = slice(j * MCH, (j + 1) * MCH)
                for h in range(2):
                    nc.tensor.matmul(
                        ptk[0:1, 1, sl], ones[:], acc2[:, h, sl],
                        start=(h == 0), stop=False,
                    )
                for h in range(2):
                    nc.tensor.matmul(
                        ptk[0:1, 1, sl], s31_sb[:, h:h + 1], u_last[:, h, sl],
                        start=False, stop=(h == 1),
                    )
                if j == 0:
                    nc.vector.tensor_copy(fin[:, sl], ptk[0:1, 1, sl])
                else:
                    nc.scalar.copy(fin[:, sl], ptk[0:1, 1, sl])
            nc.sync.dma_start(out_d[:], fin[:])

    nc.compile()
    return nc


_CACHE = {}


def _get_nc(pair_ops):
    if pair_ops not in _CACHE:
        _CACHE[pair_ops] = build(pair_ops)
    return _CACHE[pair_ops]


def kernel(x, x_train, gamma, weight, bias):
    global LAST_RESULTS
    x = np.asarray(x, dtype=np.float32)
    x_train = np.asarray(x_train, dtype=np.float32)
    g = float(np.asarray(gamma).reshape(-1)[0])
    w = np.asarray(weight, dtype=np.float32).reshape(N)
    b = np.float32(np.asarray(bias).reshape(-1)[0])

    P = 128
    KT = D // P
    NT = N // P
    NPAIR = NT // 2
    xx = np.einsum("md,md->m", x, x, dtype=np.float64, optimize=True)
    yy = np.einsum("nd,nd->n", x_train, x_train, dtype=np.float64, optimize=True)

    # Combined per-n exp offset: c_n = -g*||xt_n||^2 + ln|w_n|. Sort n by
    # (sign(w), c) and interleave so activation pair i, partition p holds
    # sorted slots 256i+2p (tile 2i) and 256i+2p+1 (tile 2i+1): pairs
    # share a midpoint bias and are sign-pure except one boundary pair.
    c = -g * yy + np.log(np.maximum(np.abs(w), 1e-30))
    neg = (w < 0).astype(np.int64)
    order = np.lexsort((c, neg))  # positives (by c), then negatives (by c)
    npos = int((neg == 0).sum())

    slot = order.reshape(NPAIR, P, 2)             # [pair, partition, h]
    idx = slot.transpose(0, 2, 1).reshape(NT, P)  # [tile, partition]

    bn = np.ascontiguousarray(
        c[slot].mean(axis=2).T.astype(np.float32)  # [128, NPAIR]
    )

    # Per-pair accumulate ops + sign columns for the mixed boundary pair.
    # The last pair is always 'f': its u planes feed the final matmuls
    # directly with the s31 sign columns as the stationary operand.
    pair_ops = []
    bp = npos // 256
    for i in range(NPAIR):
        if i == NPAIR - 1:
            pair_ops.append("f")
        elif (i + 1) * 256 <= npos:
            pair_ops.append("a")
        elif i * 256 >= npos:
            pair_ops.append("s")
        else:
            pair_ops.append("m")
    pair_ops = tuple(pair_ops)
    sg = np.ones((P, 2), dtype=np.float32)
    if "m" in pair_ops:
        sg = np.where(w[slot[bp]] >= 0, 1.0, -1.0).astype(np.float32)
    sg = np.ascontiguousarray(sg)
    s31 = np.ascontiguousarray(
        np.where(w[slot[NPAIR - 1]] >= 0, 1.0, -1.0).astype(np.float16)
    )

    # fp8 operand layouts, partition-major for single-dispatch DMA:
    # arr[p, k, cols] = src[k*128 + p, cols].
    xt_t = np.ascontiguousarray(x.T).astype(ml_dtypes.float8_e4m3)  # [D, M]
    xt_r = xt_t.reshape(KT, P, M).transpose(1, 0, 2)                # [P, KT, M]
    bt_t = ((2.0 * g) * x_train[idx.reshape(-1)].T).astype(ml_dtypes.float8_e4m3)
    # [P, NT, KT, P]: per x_train-group DMA is fully contiguous per partition
    bt_r = np.ascontiguousarray(
        bt_t.reshape(KT, P, NT, P).transpose(1, 2, 0, 3)
    )

    in_maps = []
    for cix in range(NCORES):
        sl = slice(cix * MC, (cix + 1) * MC)
        in_maps.append({
            "xt": np.ascontiguousarray(xt_r[:, :, sl]),
            "bt": bt_r,
            "bn": bn,
            "sg": sg,
            "s31": s31,
        })

    nc = _get_nc(pair_ops)
    res = run_bass_kernel_spmd(nc, in_maps, core_ids=list(range(NCORES)))
    LAST_RESULTS = res
    s = np.concatenate(
        [np.asarray(res.results[c]["out"], dtype=np.float32).reshape(MC) for c in range(NCORES)]
    )
    # Epilogue: out = S * exp(-g*||x||^2) + b
    out = (s * np.exp(-g * xx)).astype(np.float32) + b
    return out.astype(np.float32).reshape(M, 1)


# revision 36
# speedup vs baseline: 174.8408x; 174.8385x over previous
"""RBF-kernel SVM inference on 8 Trainium2 NeuronCores.

out[m] = sum_n w[n] * exp(-g * ||x[m] - xt[n]||^2) + b
       = exp(-g*||x[m]||^2) * sum_n w[n] * exp(2g*x[m].xt[n] - g*||xt[n]||^2) + b

Sharding: rows of x split across 8 cores (1024 each); x_train / weight
replicated.

Per core, the [8192, 1024] kernel slab is produced as 32 "pairs" of
128-wide n-tiles: 8 fp8 DoubleRow matmuls (K=512) fill a [128, 2, 1024]
PSUM tile, one 2048-wide Exp on ScalarE produces fp16
u = |w_n| * exp(2g*x.xt_n - g*||xt_n||^2)  (ln|w| and -g*||xt||^2 are
both folded into the per-partition activation bias), and a single
VectorE tensor add/subtract accumulates the pair into acc2. The
n-points are sorted host-side by (sign(w), combined bias) so that (a)
each pair shares one bias vector — enabling the 2048-wide activation —
and (b) every pair is sign-pure (one mixed boundary pair uses
scalar_tensor_tensor with a +-1 column instead). The final 128-way
cross-partition sum is eight K=128 matmuls (the last pair's u planes
feed theirs directly, signs as the stationary operand, keeping the
last VectorE op off the tail); the host applies exp(-g*||x||^2) and
the bias to the returned sums.

ScalarE is the bottleneck engine (32 x 1.97us back-to-back Exps =
63us of the ~84us total); PE (~57us) and DVE (~41us) hide under it.
Junk matmuls at the head warm the PE HAM clock gate while the input
DMAs land on all three DGE rings.
"""

import os
import sys

for _p in ("/opt/trn_rl_repo", "/root/.axon_site/_ro/trn_rl_repo"):
    if os.path.isdir(_p) and _p not in sys.path:
        sys.path.append(_p)

import numpy as np
import ml_dtypes

import concourse.bass as bass
import concourse.mybir as mybir
import concourse.tile as tile
from concourse import bacc
from concourse.bass_utils import run_bass_kernel_spmd

M, N, D = 8192, 8192, 512
NCORES = 8
MC = M // NCORES  # rows of x per core

LAST_RESULTS = None  # BassKernelResults of the most recent run (for test.py)

# x_train DMA groups (n-tile counts): tiny leading groups so the first
# matmuls can start after ~128KB, then big single-dispatch chunks.
GROUPS = (1, 1, 6, 8, 16, 32)
PREWARM = 6   # junk N=512 matmuls at the head to lift the PE HAM clock gate


def build(pair_ops, mc=MC, n=N, d=D, ncores=NCORES):
    """Build + compile the per-core program.

    pair_ops: tuple of 32 chars, one per n-tile pair: 'a' = acc2 += u,
    's' = acc2 -= u, 'm' = mixed-sign pair (uses the sg +-1 columns).
    """
    P = 128
    KT = d // P            # K tiles in the contraction (4)
    NT = n // P            # n tiles (64)
    NPAIR = NT // 2        # activation pairs (32)
    KP = KT // 2           # DoubleRow passes (K=256 each)
    MCH = 512              # m chunk = one PSUM bank of f32
    assert len(pair_ops) == NPAIR

    f32 = mybir.dt.float32
    bf16 = mybir.dt.bfloat16
    f16 = mybir.dt.float16
    f8 = mybir.dt.float8e4

    nc = bacc.Bacc(
        "TRN2",
        target_bir_lowering=False,
        debug=False,
        enable_asserts=False,
        num_devices=ncores,
    )

    # Host-prepared layouts: leading dim is the SBUF partition.
    xt_d = nc.dram_tensor("xt", (P, KT, mc), f8, kind="ExternalInput")
    bt_d = nc.dram_tensor("bt", (P, NT, KT, P), f8, kind="ExternalInput")
    bn_d = nc.dram_tensor("bn", (P, NPAIR), f32, kind="ExternalInput")
    sg_d = nc.dram_tensor("sg", (P, 2), f32, kind="ExternalInput")
    s31_d = nc.dram_tensor("s31", (P, 2), f16, kind="ExternalInput")
    out_d = nc.dram_tensor("out", (1, mc), f32, kind="ExternalOutput")

    EXP = mybir.ActivationFunctionType.Exp
    MUL = mybir.AluOpType.mult
    ADD = mybir.AluOpType.add

    gstart = np.cumsum([0] + list(GROUPS))

    with tile.TileContext(nc) as tc:
        with (
            tc.tile_pool(name="const", bufs=1) as const,
            tc.tile_pool(name="bt_pool", bufs=1) as bt_pool,
            tc.tile_pool(name="e_pool", bufs=3) as e_pool,
            tc.tile_pool(name="pt_pool", bufs=2, space="PSUM") as pt_pool,
        ):
            bn_sb = const.tile([P, NPAIR], f32, name="bn_sb")
            sg_sb = const.tile([P, 2], f32, name="sg_sb")
            s31_sb = const.tile([P, 2], f16, name="s31_sb")
            xt_sb = const.tile([P, KT, mc], f8, name="xt_sb")
            ones = const.tile([P, 1], f16, name="ones")
            acc2 = const.tile([P, 2, mc], f16, name="acc2")
            jw = const.tile([P, 2, P], f8, name="jw")
            jm = const.tile([P, 2, MCH], f8, name="jm")

            # DMA dispatches first so transfers start immediately: the x
            # slab is quartered across the SP and Activation HW-DGE rings
            # (K-pass 0 quarters lead), x_train streams on the gpsimd
            # software DGE. One early dispatch on the Scalar queue is
            # harmless — activations start much later.
            nc.sync.dma_start(xt_sb[:, 0:1, :], xt_d[:, 0:1, :])
            nc.scalar.dma_start(xt_sb[:, 1:2, :], xt_d[:, 1:2, :])
            nc.sync.dma_start(xt_sb[:, 2:3, :], xt_d[:, 2:3, :])
            nc.scalar.dma_start(xt_sb[:, 3:4, :], xt_d[:, 3:4, :])
            nc.sync.dma_start(bn_sb[:], bn_d[:])
            nc.sync.dma_start(sg_sb[:], sg_d[:])
            nc.sync.dma_start(s31_sb[:], s31_d[:])

            bt_sb = []
            for gi, gn in enumerate(GROUPS):
                t = bt_pool.tile([P, gn, KT, P], f8, name=f"bt_sb{gi}")
                bt_sb.append(t)
                nc.gpsimd.dma_start(
                    t[:], bt_d[:, gstart[gi]:gstart[gi + 1], :, :]
                )

            # PE warm-up on junk operands while the DMAs land: keeps the
            # HAM activity window busy so the real matmul stream starts at
            # 2.4 GHz instead of 1.2.
            nc.vector.memset(jw[:], 0.0)
            nc.vector.memset(jm[:], 0.0)
            nc.vector.memset(ones[:], 1.0)
            nc.vector.memset(acc2[:], 0.0)
            if PREWARM:
                ptw = pt_pool.tile([P, 2, mc], f32, name="ptw", tag="pt")
                for r in range(PREWARM):
                    nc.tensor.matmul(
                        ptw[:, 0, 0:MCH], jw[:], jm[:],
                        perf_mode=mybir.MatmulPerfMode.DoubleRow,
                    )

            def bt_slice(t, p):
                """Stationary operand for n-tile t, DoubleRow pass p."""
                gi = int(np.searchsorted(gstart, t, side="right")) - 1
                lc = t - gstart[gi]
                return bt_sb[gi][:, lc, 2 * p:2 * p + 2, :]

            u_last = None
            for i in range(NPAIR):
                pt2 = pt_pool.tile([P, 2, mc], f32, name="pt2", tag="pt")
                for p in range(KP):
                    for h in range(2):
                        t = 2 * i + h
                        for j in range(mc // MCH):
                            nc.tensor.matmul(
                                pt2[:, h, j * MCH:(j + 1) * MCH],
                                bt_slice(t, p),
                                xt_sb[:, 2 * p:2 * p + 2, j * MCH:(j + 1) * MCH],
                                start=(p == 0),
                                stop=(p == KP - 1),
                                perf_mode=mybir.MatmulPerfMode.DoubleRow,
                            )
                u = e_pool.tile([P, 2, mc], f16, name="u")
                nc.scalar.activation(u[:], pt2[:], EXP, bias=bn_sb[:, i:i + 1])
                op = pair_ops[i]
                if op == "f":
                    # Last pair: consumed directly by the final matmuls
                    # (signs come in as the s31 stationary columns), keeping
                    # the VectorE accumulate off the tail critical path.
                    u_last = u
                elif op == "a":
                    nc.vector.tensor_add(acc2[:], acc2[:], u[:])
                elif op == "s":
                    nc.vector.tensor_sub(acc2[:], acc2[:], u[:])
                else:  # mixed-sign boundary pair
                    for h in range(2):
                        nc.vector.scalar_tensor_tensor(
                            acc2[:, h, :], u[:, h, :], sg_sb[:, h:h + 1],
                            acc2[:, h, :], MUL, ADD,
                        )

            # 128-way cross-partition sum into ptk's upper plane (ptk
            # recycles the pool slot last read by ACT pair 30, so the
            # acc2 sums run DURING the last activation; the last pair's u
            # feeds its two matmuls directly, gated only on that
            # activation). S = ones.T @ acc2 planes + s31.T @ u_last
            # planes, accumulated per chunk; host applies exp(-g*||x||^2)
            # and the bias.
            ptk = pt_pool.tile([P, 2, mc], f32, name="ptk", tag="pt")
            fin = const.tile([1, mc], f32, name="fin")
            for j in range(mc // MCH):
                sl = slice(j * MCH, (j + 1) * MCH)
                for h in range(2):
                    nc.tensor.matmul(
                        ptk[0:1, 1, sl], ones[:], acc2[:, h, sl],
                        start=(h == 0), stop=False,
                    )
                for h in range(2):
                    nc.tensor.matmul(
                        ptk[0:1, 1, sl], s31_sb[:, h:h + 1], u_last[:, h, sl],
                        start=False, stop=(h == 1),
                    )
                if j == 0:
                    nc.vector.tensor_copy(fin[:, sl], ptk[0:1, 1, sl])
                else:
                    nc.scalar.copy(fin[:, sl], ptk[0:1, 1, sl])
            nc.sync.dma_start(out_d[:], fin[:])

    nc.compile()
    return nc


_CACHE = {}


def _get_nc(pair_ops):
    if pair_ops not in _CACHE:
        _CACHE[pair_ops] = build(pair_ops)
    return _CACHE[pair_ops]


def kernel(x, x_train, gamma, weight, bias):
    global LAST_RESULTS
    x = np.asarray(x, dtype=np.float32)
    x_train = np.asarray(x_train, dtype=np.float32)
    g = float(np.asarray(gamma).reshape(-1)[0])
    w = np.asarray(weight, dtype=np.float32).reshape(N)
    b = np.float32(np.asarray(bias).reshape(-1)[0])

    P = 128
    KT = D // P
    NT = N // P
    NPAIR = NT // 2
    xx = np.einsum("md,md->m", x, x, dtype=np.float64, optimize=True)
    yy = np.einsum("nd,nd->n", x_train, x_train, dtype=np.float64, optimize=True)

    # Combined per-n exp offset: c_n = -g*||xt_n||^2 + ln|w_n|. Sort n by
    # (sign(w), c) and interleave so activation pair i, partition p holds
    # sorted slots 256i+2p (tile 2i) and 256i+2p+1 (tile 2i+1): pairs
    # share a midpoint bias and are sign-pure except one boundary pair.
    c = -g * yy + np.log(np.maximum(np.abs(w), 1e-30))
    neg = (w < 0).astype(np.int64)
    order = np.lexsort((c, neg))  # positives (by c), then negatives (by c)
    npos = int((neg == 0).sum())

    slot = order.reshape(NPAIR, P, 2)             # [pair, partition, h]
    idx = slot.transpose(0, 2, 1).reshape(NT, P)  # [tile, partition]

    bn = np.ascontiguousarray(
        c[slot].mean(axis=2).T.astype(np.float32)  # [128, NPAIR]
    )

    # Per-pair accumulate ops + sign columns for the mixed boundary pair.
    # The last pair is always 'f': its u planes feed the final matmuls
    # directly with the s31 sign columns as the stationary operand.
    pair_ops = []
    bp = npos // 256
    for i in range(NPAIR):
        if i == NPAIR - 1:
            pair_ops.append("f")
        elif (i + 1) * 256 <= npos:
            pair_ops.append("a")
        elif i * 256 >= npos:
            pair_ops.append("s")
        else:
            pair_ops.append("m")
    pair_ops = tuple(pair_ops)
    sg = np.ones((P, 2), dtype=np.float32)
    if "m" in pair_ops:
        sg = np.where(w[slot[bp]] >= 0, 1.0, -1.0).astype(np.float32)
    sg = np.ascontiguousarray(sg)
    s31 = np.ascontiguousarray(
        np.where(w[slot[NPAIR - 1]] >= 0, 1.0, -1.0).astype(np.float16)
    )

    # fp8 operand layouts, partition-major for single-dispatch DMA:
    # arr[p, k, cols] = src[k*128 + p, cols].
    xt_t = np.ascontiguousarray(x.T).astype(ml_dtypes.float8_e4m3)  # [D, M]
    xt_r = xt_t.reshape(KT, P, M).transpose(1, 0, 2)                # [P, KT, M]
    bt_t = ((2.0 * g) * x_train[idx.reshape(-1)].T).astype(ml_dtypes.float8_e4m3)
    # [P, NT, KT, P]: per x_train-group DMA is fully contiguous per partition
    bt_r = np.ascontiguousarray(
        bt_t.reshape(KT, P, NT, P).transpose(1, 2, 0, 3)
    )

    in_maps = []
    for cix in range(NCORES):
        sl = slice(cix * MC, (cix + 1) * MC)
        in_maps.append({
            "xt": np.ascontiguousarray(xt_r[:, :, sl]),
            "bt": bt_r,
            "bn": bn,
            "sg": sg,
            "s31": s31,
        })

    nc = _get_nc(pair_ops)
    res = run_bass_kernel_spmd(nc, in_maps, core_ids=list(range(NCORES)))
    LAST_RESULTS = res
    s = np.concatenate(
        [np.asarray(res.results[c]["out"], dtype=np.float32).reshape(MC) for c in range(NCORES)]
    )
    # Epilogue: out = S * exp(-g*||x||^2) + b
    out = (s * np.exp(-g * xx)).astype(np.float32) + b
    return out.astype(np.float32).reshape(M, 1)
